# revision 9
# baseline (speedup 1.0000x reference)
"""Dcls2dFull Trainium2 kernel.

Computes: dense 12x12 kernel from 3x3 weights scattered at learnable fractional
positions (bilinear), then conv2d(x, kernel, pad=5) + bias.

Strategy:
  - Host: build the dense kernel [64,64,12,12] in numpy (tiny), find the set of
    nonzero row/col taps (an 8x8 subset for this init), pair row-taps so each
    matmul contracts over K = 64 cin x 2 row-taps = 128.
  - Device (8 cores, data-parallel over batch): per core 4 images. SBUF holds
    the padded image twice per row-delta (base on partitions 0-63, row-shifted
    copy on 64-127) so one matmul covers a row-tap pair. Output chunks of
    5 rows x 95 cols (N=475) accumulate 32 tap-pair matmuls in PSUM; two chunks
    run concurrently via PE column tiling (tile_position (0,0)/(0,64)).
  - bf16 matmul inputs (fp32 accumulation in PSUM), fp32 output.
"""

import os
import sys
import types

import numpy as np

# ---- fixed problem geometry (from the nn.Module config) ----
B, CIN, COUT, H, W = 32, 64, 64, 96, 96
K0 = K1 = 3
D0 = D1 = 4
LIM0 = LIM1 = 12
PAD = 5
HO = H + 2 * PAD - LIM0 + 1   # 95
WO = W + 2 * PAD - LIM1 + 1   # 95
N_CORES = 8
IMG_PER_CORE = B // N_CORES   # 4
CHUNK = 5                     # output rows per matmul chunk
N_CHUNKS = HO // CHUNK        # 19

_SETUP_DONE = False
TRACE = False
LAST_EXEC_NS = None
LAST_RESULT = None


def _setup():
    global _SETUP_DONE
    if _SETUP_DONE:
        return
    if '/opt/trn_rl_repo' not in sys.path:
        sys.path.insert(0, '/opt/trn_rl_repo')

    # Register the NTFF profiling hook (the image's antenv lacks axon_hooks).
    try:
        import antenv
        if "antenv.axon_hooks" not in sys.modules:
            mod = types.ModuleType("antenv.axon_hooks")
            mod._hook = None
            def set_axon_ntff_profile_hook(h):
                mod._hook = h
            def get_axon_ntff_profile_hook():
                return mod._hook
            mod.set_axon_ntff_profile_hook = set_axon_ntff_profile_hook
            mod.get_axon_ntff_profile_hook = get_axon_ntff_profile_hook
            sys.modules["antenv.axon_hooks"] = mod
            antenv.axon_hooks = mod
            from trn_agent_boot.trn_boot import _ntff_profile_via_ctypes
            mod._hook = _ntff_profile_via_ctypes('/opt/axon/libaxon_pjrt.so')
    except Exception:
        pass

    from concourse import tile, mybir
    from concourse.vector_clock import ScopedClock

    # The pinned walrus allows a single sync-wait on the kernel-tail Drain;
    # split extra waits onto single-wait NOPs.
    def _patched(self, tick_clock, wait_clock):
        drain_inst = self.nc.sync.drain()
        wait_clock.add_sem_waits(
            drain_inst.ins, ScopedClock({None: tick_clock.global_clock}))
        si = drain_inst.ins.sync_info
        waits = list(si.on_wait) if si is not None else []
        if len(waits) > 1:
            si.on_wait[:] = waits[:1]
            for w in waits[1:]:
                nop = self.nc.sync.nop(nofuse=True, hint="drain_split")
                nop.ins.sync_info = mybir.SyncInfo(on_wait=[w], on_update=[])
        self.nc.all_engine_barrier()
        assert self.sems is not None
        popped = self.nc._tile_sem_poison_stack.pop()
        assert popped is self._sem_poison
        self.nc.clear_and_free_semaphores(list(self.sems.allocated().values()))
        self.nc.all_engine_barrier()

    tile.TileContext._drain_and_barrier = _patched
    _SETUP_DONE = True


def _split_multi_waits(nc, mybir):
    """Move extra sync-waits (>1 per instruction, unsupported by the pinned
    walrus) onto dedicated single-wait NOPs inserted just before."""
    for fn in nc.m.functions:
        for blk in fn.blocks:
            newlist = []
            for inst in blk.instructions:
                si = inst.sync_info
                if si is not None and si.on_wait is not None and len(si.on_wait) > 1:
                    waits = list(si.on_wait)
                    for w in waits[:-1]:
                        nop = mybir.InstNoOp(
                            name=f"I-{nc.next_id()}",
                            engine=inst.engine,
                            bass_nofuse=True,
                            sync_info=mybir.SyncInfo(on_wait=[w], on_update=[]),
                        )
                        newlist.append(nop)
                    si.on_wait[:] = waits[-1:]
                newlist.append(inst)
            blk.instructions[:] = newlist


def construct_dense_kernel(weight, P1, P2):
    """Numpy replica of the reference construct_kernel (float32)."""
    w = weight.astype(np.float32)
    p1 = np.clip(P1.astype(np.float32) + (LIM0 // 2), 0.0, LIM0 - 1)
    p2 = np.clip(P2.astype(np.float32) + (LIM1 // 2), 0.0, LIM1 - 1)
    f1 = np.floor(p1)
    f2 = np.floor(p2)
    r1 = p1 - f1
    r2 = p2 - f2
    i1 = f1.astype(np.int64)
    i2 = f2.astype(np.int64)
    i1p = np.minimum(i1 + 1, LIM0 - 1)
    i2p = np.minimum(i2 + 1, LIM1 - 1)
    co = np.broadcast_to(np.arange(COUT)[:, None, None, None], w.shape)
    ci = np.broadcast_to(np.arange(CIN)[None, :, None, None], w.shape)
    kern = np.zeros((COUT, CIN, LIM0, LIM1), np.float32)
    np.add.at(kern, (co, ci, i1, i2), w * (1 - r1) * (1 - r2))
    np.add.at(kern, (co, ci, i1p, i2), w * r1 * (1 - r2))
    np.add.at(kern, (co, ci, i1, i2p), w * (1 - r1) * r2)
    np.add.at(kern, (co, ci, i1p, i2p), w * r1 * r2)
    return kern


def _pair_rows(rows):
    """Pair row taps minimizing distinct deltas: greedy delta=1 first, then
    pair leftovers in sorted order (arbitrary deltas). Returns (pairs, singles)
    where pairs = [(r1, r2), ...] with r2 > r1."""
    rows = sorted(rows)
    used = set()
    pairs = []
    for r in rows:
        if r in used:
            continue
        if (r + 1) in rows and (r + 1) not in used:
            pairs.append((r, r + 1))
            used.add(r)
            used.add(r + 1)
    left = [r for r in rows if r not in used]
    singles = []
    while len(left) >= 2:
        pairs.append((left[0], left[1]))
        left = left[2:]
    singles = left
    return pairs, singles


def kernel(x, weight, P1, P2, bias):
    _setup()
    import ml_dtypes
    from concourse import bass, tile, mybir
    from concourse.bass_utils import run_bass_kernel_spmd

    F32 = mybir.dt.float32
    BF16 = mybir.dt.bfloat16

    x = np.asarray(x, np.float32)
    bias = np.asarray(bias, np.float32)

    # ---- host: dense kernel + tap structure ----
    kern = construct_dense_kernel(np.asarray(weight), np.asarray(P1), np.asarray(P2))
    nz = np.abs(kern).max(axis=(0, 1)) > 0          # [12, 12]
    rows = [int(r) for r in np.where(nz.any(axis=1))[0]]
    cols = [int(c) for c in np.where(nz.any(axis=0))[0]]
    pairs, singles = _pair_rows(rows)
    deltas = sorted({r2 - r1 for r1, r2 in pairs})
    max_c = max(cols)

    # padded geometry: SBUF row j holds x[ih = j - PAD] (base copy); reads go
    # up to j = (N_CHUNKS-1)*CHUNK + max_row_tap + CHUNK - 1.
    WPAD = max_c + WO
    WPAD += (4 - WPAD % 4) % 4                       # padded row length (aligned)
    HPAD_SB = (N_CHUNKS - 1) * CHUNK + max(rows) + CHUNK
    XP_ROWS = HPAD_SB + (max(deltas) if deltas else 0)

    # tap-pair list: (pair_idx within wp, delta, r1, c)
    taps = [(r1, r2, c) for (r1, r2) in pairs for c in cols]
    n_taps = len(taps)
    n_sing = len(singles) * len(cols)

    # ---- host tensors ----
    # wp: [128, (n_taps + n_sing) * 64] bf16 stationary weights
    wp = np.zeros((128, (n_taps + n_sing) * 64), np.float32)
    for p, (r1, r2, c) in enumerate(taps):
        wp[0:64, p * 64:(p + 1) * 64] = kern[:, :, r1, c].T
        wp[64:128, p * 64:(p + 1) * 64] = kern[:, :, r2, c].T
    for s, (r, c) in enumerate(((r, c) for r in singles for c in cols)):
        p = n_taps + s
        wp[0:64, p * 64:(p + 1) * 64] = kern[:, :, r, c].T
    wp_bf = wp.astype(ml_dtypes.bfloat16)

    bias_b = np.concatenate([bias, bias])[:, None].astype(np.float32)  # [128,1]

    # xp: [B, CIN, XP_ROWS, WPAD] bf16, zero-padded (PAD top rows, PAD left cols)
    xp = np.zeros((B, CIN, XP_ROWS, WPAD), ml_dtypes.bfloat16)
    xp[:, :, PAD:PAD + H, PAD:PAD + W] = x.astype(ml_dtypes.bfloat16)

    # ---- build device program (one SPMD program for all 8 cores) ----
    nc = bass.Bass("TRN2")
    xp_d = nc.declare_dram_parameter(
        "xp", [IMG_PER_CORE, CIN, XP_ROWS, WPAD], BF16, isOutput=False)
    wp_d = nc.declare_dram_parameter("wp", list(wp_bf.shape), BF16, isOutput=False)
    bias_d = nc.declare_dram_parameter("biasb", [128, 1], F32, isOutput=False)
    out_d = nc.declare_dram_parameter(
        "out", [IMG_PER_CORE, COUT, HO, WO], F32, isOutput=True)

    n_flat = IMG_PER_CORE * N_CHUNKS   # 76 chunks -> 38 pairs

    with tile.TileContext(nc) as tc:
        with (
            tc.tile_pool(name="const", bufs=1) as cpool,
            tc.tile_pool(name="xb", bufs=3) as xpool,
            tc.tile_pool(name="psum", bufs=8, space="PSUM") as ppool,
            tc.tile_pool(name="stage", bufs=8) as spool,
        ):
            wt = cpool.tile([128, wp_bf.shape[1]], BF16)
            # first taps' weights land first so matmuls can start immediately
            nc.sync.dma_start(wt[:, 0:4 * 64], wp_d[:, 0:4 * 64])
            bt = cpool.tile([128, 1], F32)
            nc.scalar.dma_start(bt[:], bias_d[:])
            nc.scalar.dma_start(wt[:, 4 * 64:], wp_d[:, 4 * 64:])

            xb_tiles = {}
            _eng = [0]

            def dma(dst, src):
                # alternate HWDGE issue between sync and scalar sequencers
                eng = nc.sync if _eng[0] % 2 == 0 else nc.scalar
                _eng[0] += 1
                eng.dma_start(dst, src)

            def get_xb(img):
                if img in xb_tiles:
                    return xb_tiles[img]
                bufs = {}
                tiles = {d: xpool.tile([128, HPAD_SB, WPAD], BF16,
                                       name=f"xb_{img}_{d}", tag=f"xbd{d}")
                         for d in deltas}
                # segment the row range so early chunks' matmuls can start
                # before the whole image lands in SBUF
                if img == 0:
                    segs = [(0, 10), (10, 18), (18, 34), (34, 68), (68, HPAD_SB)]
                else:
                    segs = [(0, 52), (52, HPAD_SB)]
                for s0, s1 in segs:
                    for d in deltas:
                        t = tiles[d]
                        # top half: base padded image rows [s0:s1)
                        dma(t[0:64, s0:s1, :], xp_d[img, :, s0:s1, :])
                        # bottom half: rows shifted by +d
                        dma(t[64:128, s0:s1, :], xp_d[img, :, s0 + d:s1 + d, :])
                bufs.update(tiles)
                if singles:
                    t = xpool.tile([64, HPAD_SB, WPAD], BF16, tag="xbs")
                    nc.sync.dma_start(t[:, :, :], xp_d[img, :, 0:HPAD_SB, :])
                    bufs["single"] = t
                xb_tiles[img] = bufs
                # drop oldest entries beyond the pool depth to keep dict small
                return bufs

            for t2 in range(n_flat // 2):
                ca, cb = 2 * t2, 2 * t2 + 1
                img_a, ch_a = divmod(ca, N_CHUNKS)
                img_b, ch_b = divmod(cb, N_CHUNKS)
                a0, b0 = ch_a * CHUNK, ch_b * CHUNK
                xa = get_xb(img_a)
                xb = get_xb(img_b)

                ps = ppool.tile([128, CHUNK, WO], F32)
                for p, (r1, r2, c) in enumerate(taps):
                    d = r2 - r1
                    lhsT = wt[:, p * 64:(p + 1) * 64]
                    nc.tensor.matmul(
                        ps[0:64, :, :], lhsT,
                        xa[d][:, a0 + r1:a0 + r1 + CHUNK, c:c + WO],
                        start=(p == 0), stop=(p == n_taps - 1 and not singles),
                        tile_position=(0, 0),
                    )
                    nc.tensor.matmul(
                        ps[64:128, :, :], lhsT,
                        xb[d][:, b0 + r1:b0 + r1 + CHUNK, c:c + WO],
                        start=(p == 0), stop=(p == n_taps - 1 and not singles),
                        tile_position=(0, 64),
                    )
                if singles:
                    si = 0
                    for r in singles:
                        for c in cols:
                            p = n_taps + si
                            lhsT = wt[0:64, p * 64:(p + 1) * 64]
                            last = (si == n_sing - 1)
                            nc.tensor.matmul(
                                ps[0:64, :, :], lhsT,
                                xa["single"][:, a0 + r:a0 + r + CHUNK, c:c + WO],
                                start=False, stop=last, tile_position=(0, 0),
                            )
                            nc.tensor.matmul(
                                ps[64:128, :, :], lhsT,
                                xb["single"][:, b0 + r:b0 + r + CHUNK, c:c + WO],
                                start=False, stop=last, tile_position=(0, 64),
                            )
                            si += 1

                st = spool.tile([128, CHUNK, WO], F32)
                nc.vector.tensor_scalar_add(st[:, :, :], ps[:, :, :], bt[:])
                nc.sync.dma_start(out_d[img_a, :, a0:a0 + CHUNK, :], st[0:64, :, :])
                nc.sync.dma_start(out_d[img_b, :, b0:b0 + CHUNK, :], st[64:128, :, :])

    _split_multi_waits(nc, mybir)

    # ---- shard, run, gather ----
    in_maps = []
    for core in range(N_CORES):
        sl = slice(core * IMG_PER_CORE, (core + 1) * IMG_PER_CORE)
        in_maps.append({
            "xp": np.ascontiguousarray(xp[sl]),
            "wp": wp_bf,
            "biasb": bias_b,
        })

    global LAST_EXEC_NS, LAST_RESULT
    res = run_bass_kernel_spmd(
        nc, in_maps, core_ids=list(range(N_CORES)), trace=TRACE)
    LAST_RESULT = res
    LAST_EXEC_NS = res.exec_time_ns

    out = np.concatenate([res.results[c]["out"] for c in range(N_CORES)], axis=0)
    return out.astype(np.float32)


# revision 10
# speedup vs baseline: 1.0105x; 1.0105x over previous
"""Dcls2dFull Trainium2 kernel.

Computes: dense 12x12 kernel from 3x3 weights scattered at learnable fractional
positions (bilinear), then conv2d(x, kernel, pad=5) + bias.

Strategy:
  - Host: build the dense kernel [64,64,12,12] in numpy (tiny), find the set of
    nonzero row/col taps (an 8x8 subset for this init), pair row-taps so each
    matmul contracts over K = 64 cin x 2 row-taps = 128.
  - Device (8 cores, data-parallel over batch): per core 4 images. SBUF holds
    the padded image twice per row-delta (base on partitions 0-63, row-shifted
    copy on 64-127) so one matmul covers a row-tap pair. Output chunks of
    5 rows x 95 cols (N=475) accumulate 32 tap-pair matmuls in PSUM; two chunks
    run concurrently via PE column tiling (tile_position (0,0)/(0,64)).
  - bf16 matmul inputs (fp32 accumulation in PSUM), fp32 output.
"""

import os
import sys
import types

import numpy as np

# ---- fixed problem geometry (from the nn.Module config) ----
B, CIN, COUT, H, W = 32, 64, 64, 96, 96
K0 = K1 = 3
D0 = D1 = 4
LIM0 = LIM1 = 12
PAD = 5
HO = H + 2 * PAD - LIM0 + 1   # 95
WO = W + 2 * PAD - LIM1 + 1   # 95
N_CORES = 8
IMG_PER_CORE = B // N_CORES   # 4
CHUNK = 5                     # output rows per matmul chunk
N_CHUNKS = HO // CHUNK        # 19

_SETUP_DONE = False
TRACE = False
LAST_EXEC_NS = None
LAST_RESULT = None


def _setup():
    global _SETUP_DONE
    if _SETUP_DONE:
        return
    if '/opt/trn_rl_repo' not in sys.path:
        sys.path.insert(0, '/opt/trn_rl_repo')

    # Register the NTFF profiling hook (the image's antenv lacks axon_hooks).
    try:
        import antenv
        if "antenv.axon_hooks" not in sys.modules:
            mod = types.ModuleType("antenv.axon_hooks")
            mod._hook = None
            def set_axon_ntff_profile_hook(h):
                mod._hook = h
            def get_axon_ntff_profile_hook():
                return mod._hook
            mod.set_axon_ntff_profile_hook = set_axon_ntff_profile_hook
            mod.get_axon_ntff_profile_hook = get_axon_ntff_profile_hook
            sys.modules["antenv.axon_hooks"] = mod
            antenv.axon_hooks = mod
            from trn_agent_boot.trn_boot import _ntff_profile_via_ctypes
            mod._hook = _ntff_profile_via_ctypes('/opt/axon/libaxon_pjrt.so')
    except Exception:
        pass

    from concourse import tile, mybir
    from concourse.vector_clock import ScopedClock

    # The pinned walrus allows a single sync-wait on the kernel-tail Drain;
    # split extra waits onto single-wait NOPs.
    def _patched(self, tick_clock, wait_clock):
        drain_inst = self.nc.sync.drain()
        wait_clock.add_sem_waits(
            drain_inst.ins, ScopedClock({None: tick_clock.global_clock}))
        si = drain_inst.ins.sync_info
        waits = list(si.on_wait) if si is not None else []
        if len(waits) > 1:
            si.on_wait[:] = waits[:1]
            for w in waits[1:]:
                nop = self.nc.sync.nop(nofuse=True, hint="drain_split")
                nop.ins.sync_info = mybir.SyncInfo(on_wait=[w], on_update=[])
        self.nc.all_engine_barrier()
        assert self.sems is not None
        popped = self.nc._tile_sem_poison_stack.pop()
        assert popped is self._sem_poison
        self.nc.clear_and_free_semaphores(list(self.sems.allocated().values()))
        self.nc.all_engine_barrier()

    tile.TileContext._drain_and_barrier = _patched
    _SETUP_DONE = True


def _split_multi_waits(nc, mybir):
    """Move extra sync-waits (>1 per instruction, unsupported by the pinned
    walrus) onto dedicated single-wait NOPs inserted just before."""
    for fn in nc.m.functions:
        for blk in fn.blocks:
            newlist = []
            for inst in blk.instructions:
                si = inst.sync_info
                if si is not None and si.on_wait is not None and len(si.on_wait) > 1:
                    waits = list(si.on_wait)
                    for w in waits[:-1]:
                        nop = mybir.InstNoOp(
                            name=f"I-{nc.next_id()}",
                            engine=inst.engine,
                            bass_nofuse=True,
                            sync_info=mybir.SyncInfo(on_wait=[w], on_update=[]),
                        )
                        newlist.append(nop)
                    si.on_wait[:] = waits[-1:]
                newlist.append(inst)
            blk.instructions[:] = newlist


def construct_dense_kernel(weight, P1, P2):
    """Numpy replica of the reference construct_kernel (float32)."""
    w = weight.astype(np.float32)
    p1 = np.clip(P1.astype(np.float32) + (LIM0 // 2), 0.0, LIM0 - 1)
    p2 = np.clip(P2.astype(np.float32) + (LIM1 // 2), 0.0, LIM1 - 1)
    f1 = np.floor(p1)
    f2 = np.floor(p2)
    r1 = p1 - f1
    r2 = p2 - f2
    i1 = f1.astype(np.int64)
    i2 = f2.astype(np.int64)
    i1p = np.minimum(i1 + 1, LIM0 - 1)
    i2p = np.minimum(i2 + 1, LIM1 - 1)
    co = np.broadcast_to(np.arange(COUT)[:, None, None, None], w.shape)
    ci = np.broadcast_to(np.arange(CIN)[None, :, None, None], w.shape)
    kern = np.zeros((COUT, CIN, LIM0, LIM1), np.float32)
    np.add.at(kern, (co, ci, i1, i2), w * (1 - r1) * (1 - r2))
    np.add.at(kern, (co, ci, i1p, i2), w * r1 * (1 - r2))
    np.add.at(kern, (co, ci, i1, i2p), w * (1 - r1) * r2)
    np.add.at(kern, (co, ci, i1p, i2p), w * r1 * r2)
    return kern


def _pair_rows(rows):
    """Pair row taps minimizing distinct deltas: greedy delta=1 first, then
    pair leftovers in sorted order (arbitrary deltas). Returns (pairs, singles)
    where pairs = [(r1, r2), ...] with r2 > r1."""
    rows = sorted(rows)
    used = set()
    pairs = []
    for r in rows:
        if r in used:
            continue
        if (r + 1) in rows and (r + 1) not in used:
            pairs.append((r, r + 1))
            used.add(r)
            used.add(r + 1)
    left = [r for r in rows if r not in used]
    singles = []
    while len(left) >= 2:
        pairs.append((left[0], left[1]))
        left = left[2:]
    singles = left
    return pairs, singles


def kernel(x, weight, P1, P2, bias):
    _setup()
    import ml_dtypes
    from concourse import bass, tile, mybir
    from concourse.bass_utils import run_bass_kernel_spmd

    F32 = mybir.dt.float32
    BF16 = mybir.dt.bfloat16

    x = np.asarray(x, np.float32)
    bias = np.asarray(bias, np.float32)

    # ---- host: dense kernel + tap structure ----
    kern = construct_dense_kernel(np.asarray(weight), np.asarray(P1), np.asarray(P2))
    nz = np.abs(kern).max(axis=(0, 1)) > 0          # [12, 12]
    rows = [int(r) for r in np.where(nz.any(axis=1))[0]]
    cols = [int(c) for c in np.where(nz.any(axis=0))[0]]
    pairs, singles = _pair_rows(rows)
    deltas = sorted({r2 - r1 for r1, r2 in pairs})
    max_c = max(cols)

    # padded geometry: SBUF row j holds x[ih = j - PAD] (base copy); reads go
    # up to j = (N_CHUNKS-1)*CHUNK + max_row_tap + CHUNK - 1.
    WPAD = max_c + WO
    WPAD += (4 - WPAD % 4) % 4                       # padded row length (aligned)
    HPAD_SB = (N_CHUNKS - 1) * CHUNK + max(rows) + CHUNK
    XP_ROWS = HPAD_SB + (max(deltas) if deltas else 0)

    # tap-pair list: (pair_idx within wp, delta, r1, c)
    taps = [(r1, r2, c) for (r1, r2) in pairs for c in cols]
    n_taps = len(taps)
    n_sing = len(singles) * len(cols)

    # ---- host tensors ----
    # wp: [128, (n_taps + n_sing) * 64] bf16 stationary weights
    wp = np.zeros((128, (n_taps + n_sing) * 64), np.float32)
    for p, (r1, r2, c) in enumerate(taps):
        wp[0:64, p * 64:(p + 1) * 64] = kern[:, :, r1, c].T
        wp[64:128, p * 64:(p + 1) * 64] = kern[:, :, r2, c].T
    for s, (r, c) in enumerate(((r, c) for r in singles for c in cols)):
        p = n_taps + s
        wp[0:64, p * 64:(p + 1) * 64] = kern[:, :, r, c].T
    wp_bf = wp.astype(ml_dtypes.bfloat16)

    bias_b = np.concatenate([bias, bias])[:, None].astype(np.float32)  # [128,1]

    # xp: [B, CIN, XP_ROWS, WPAD] bf16, zero-padded (PAD top rows, PAD left cols)
    xp = np.zeros((B, CIN, XP_ROWS, WPAD), ml_dtypes.bfloat16)
    xp[:, :, PAD:PAD + H, PAD:PAD + W] = x.astype(ml_dtypes.bfloat16)

    # ---- build device program (one SPMD program for all 8 cores) ----
    nc = bass.Bass("TRN2")
    xp_d = nc.declare_dram_parameter(
        "xp", [IMG_PER_CORE, CIN, XP_ROWS, WPAD], BF16, isOutput=False)
    wp_d = nc.declare_dram_parameter("wp", list(wp_bf.shape), BF16, isOutput=False)
    bias_d = nc.declare_dram_parameter("biasb", [128, 1], F32, isOutput=False)
    out_d = nc.declare_dram_parameter(
        "out", [IMG_PER_CORE, COUT, HO, WO], F32, isOutput=True)

    n_flat = IMG_PER_CORE * N_CHUNKS   # 76 chunks -> 38 pairs

    with tile.TileContext(nc) as tc:
        with (
            tc.tile_pool(name="const", bufs=1) as cpool,
            tc.tile_pool(name="xb", bufs=2) as xpool,
            tc.tile_pool(name="psum", bufs=8, space="PSUM") as ppool,
            tc.tile_pool(name="stage", bufs=8) as spool,
        ):
            wt = cpool.tile([128, wp_bf.shape[1]], BF16)
            # first taps' weights land first so matmuls can start immediately
            nc.sync.dma_start(wt[:, 0:4 * 64], wp_d[:, 0:4 * 64])
            bt = cpool.tile([128, 1], F32)
            nc.scalar.dma_start(bt[:], bias_d[:])
            nc.scalar.dma_start(wt[:, 4 * 64:], wp_d[:, 4 * 64:])

            xb_tiles = {}
            _eng = [0]

            def dma(dst, src):
                # alternate HWDGE issue between sync and scalar sequencers
                eng = nc.sync if _eng[0] % 2 == 0 else nc.scalar
                _eng[0] += 1
                eng.dma_start(dst, src)

            def get_xb(img):
                if img in xb_tiles:
                    return xb_tiles[img]
                bufs = {}
                tiles = {d: xpool.tile([128, HPAD_SB, WPAD], BF16,
                                       name=f"xb_{img}_{d}", tag=f"xbd{d}")
                         for d in deltas}
                # segment the row range so early chunks' matmuls can start
                # before the whole image lands in SBUF
                if img == 0:
                    segs = [(0, 10), (10, 18), (18, 34), (34, 68), (68, HPAD_SB)]
                else:
                    segs = [(0, 52), (52, HPAD_SB)]
                for s0, s1 in segs:
                    for d in deltas:
                        t = tiles[d]
                        # top half: base padded image rows [s0:s1)
                        dma(t[0:64, s0:s1, :], xp_d[img, :, s0:s1, :])
                        # bottom half: rows shifted by +d
                        dma(t[64:128, s0:s1, :], xp_d[img, :, s0 + d:s1 + d, :])
                bufs.update(tiles)
                if singles:
                    t = xpool.tile([64, HPAD_SB, WPAD], BF16, tag="xbs")
                    nc.sync.dma_start(t[:, :, :], xp_d[img, :, 0:HPAD_SB, :])
                    bufs["single"] = t
                xb_tiles[img] = bufs
                # drop oldest entries beyond the pool depth to keep dict small
                return bufs

            for t2 in range(n_flat // 2):
                ca, cb = 2 * t2, 2 * t2 + 1
                img_a, ch_a = divmod(ca, N_CHUNKS)
                img_b, ch_b = divmod(cb, N_CHUNKS)
                a0, b0 = ch_a * CHUNK, ch_b * CHUNK
                xa = get_xb(img_a)
                xb = get_xb(img_b)

                ps = ppool.tile([128, CHUNK, WO], F32)
                for p, (r1, r2, c) in enumerate(taps):
                    d = r2 - r1
                    lhsT = wt[:, p * 64:(p + 1) * 64]
                    nc.tensor.matmul(
                        ps[0:64, :, :], lhsT,
                        xa[d][:, a0 + r1:a0 + r1 + CHUNK, c:c + WO],
                        start=(p == 0), stop=(p == n_taps - 1 and not singles),
                        tile_position=(0, 0),
                    )
                    nc.tensor.matmul(
                        ps[64:128, :, :], lhsT,
                        xb[d][:, b0 + r1:b0 + r1 + CHUNK, c:c + WO],
                        start=(p == 0), stop=(p == n_taps - 1 and not singles),
                        tile_position=(0, 64),
                    )
                if singles:
                    si = 0
                    for r in singles:
                        for c in cols:
                            p = n_taps + si
                            lhsT = wt[0:64, p * 64:(p + 1) * 64]
                            last = (si == n_sing - 1)
                            nc.tensor.matmul(
                                ps[0:64, :, :], lhsT,
                                xa["single"][:, a0 + r:a0 + r + CHUNK, c:c + WO],
                                start=False, stop=last, tile_position=(0, 0),
                            )
                            nc.tensor.matmul(
                                ps[64:128, :, :], lhsT,
                                xb["single"][:, b0 + r:b0 + r + CHUNK, c:c + WO],
                                start=False, stop=last, tile_position=(0, 64),
                            )
                            si += 1

                st = spool.tile([128, CHUNK, WO], F32)
                nc.vector.tensor_scalar_add(st[:, :, :], ps[:, :, :], bt[:])
                nc.sync.dma_start(out_d[img_a, :, a0:a0 + CHUNK, :], st[0:64, :, :])
                nc.sync.dma_start(out_d[img_b, :, b0:b0 + CHUNK, :], st[64:128, :, :])

    _split_multi_waits(nc, mybir)

    # ---- shard, run, gather ----
    in_maps = []
    for core in range(N_CORES):
        sl = slice(core * IMG_PER_CORE, (core + 1) * IMG_PER_CORE)
        in_maps.append({
            "xp": np.ascontiguousarray(xp[sl]),
            "wp": wp_bf,
            "biasb": bias_b,
        })

    global LAST_EXEC_NS, LAST_RESULT
    res = run_bass_kernel_spmd(
        nc, in_maps, core_ids=list(range(N_CORES)), trace=TRACE)
    LAST_RESULT = res
    LAST_EXEC_NS = res.exec_time_ns

    out = np.concatenate([res.results[c]["out"] for c in range(N_CORES)], axis=0)
    return out.astype(np.float32)
